# revision 1
# baseline (speedup 1.0000x reference)
"""Trainium2 Bass kernel for nn_AttnNeck (B=4, C=256, H=W=64) — fp8 DoubleRow rewrite.

out = gamma * (v @ softmax_n(x1^T x1)) + ref, x1 = relu(conv3x3(ref, w1)),
v = relu(conv3x3(ref, w2)). Dead conv on `inputs` skipped.

Sharding: 8 cores = 4 samples x 2 half-image column shards (odd cores 180deg
rotated; conv3x3/SAME commutes with rot180).

All heavy matmuls are fp8e4m3 DoubleRow (0.5 cyc/row, 2 K-tiles per
instruction):
- convs contract (2 ic x 9 taps) as 9 DR matmuls per 512-pixel row block,
  reading dx-shifted zero-padded input copies (flat 3-dim APs). Inputs and
  weights are pre-scaled (SR=32, SW=2048) on the host to clear fp8's
  subnormal range; the PSUM descale rides the relu's activation scale.
  conv2 optionally adds an input-residual stream (rl8) and weight-residual
  stream for accuracy.
- scores: per [128n x 512m] PSUM tile, one DR matmul x8^T x8 plus a
  1-partition DR bias matmul adding -(d_m) as 2 exact fp8 pieces, so ACT
  exp reads PSUM directly (no vector shift). d = sum_c x8^2 from fp8
  squares + gpsimd partition_all_reduce (only overflow-guard accuracy
  needed; the shift cancels in softmax).
- E = exp(S - d) in fp8 (diag shift keeps E <= ~20 << 240 max).
- A = (v8 + vl8) @ E via DR (vT8/vlT8 from PE transposes); denominator
  D = ones-column DR contraction of E; 1/D broadcast via gpsimd.
"""
import sys
sys.path.insert(0, '/opt/trn_rl_repo')

import numpy as np
import ml_dtypes

B, C, H, W = 4, 256, 64, 64
HW = H * W           # 4096
MHALF = HW // 2      # 2048 columns per core
NCORES = 8
NBLK = MHALF // 512  # 4 m-blocks per core
SR, SW = 32.0, 2048.0
SCONV = 1.0 / (SR * SW)
CONV2_STREAMS = 1    # rel err: 1 -> 1.23e-2 (128us), 2 -> 8.7e-3 (156us), 3 -> 4.5e-4
F8NP = ml_dtypes.float8_e4m3

_CACHE = {}


def _build(gamma: float):
    import concourse.bacc as bacc
    import concourse.mybir as mybir
    import concourse.tile as tile
    from concourse.masks import make_identity
    from concourse.bass_isa import ReduceOp

    f32, fp8, bf16 = mybir.dt.float32, mybir.dt.float8e4, mybir.dt.bfloat16
    AF = mybir.ActivationFunctionType
    ALU = mybir.AluOpType
    DR = mybir.MatmulPerfMode.DoubleRow

    nc = bacc.Bacc("TRN2", target_bir_lowering=False, debug=False,
                   num_devices=NCORES)
    r8d = nc.dram_tensor("r8d", [128, 2, 3, 66, 64], fp8, kind="ExternalInput")
    if CONV2_STREAMS >= 2:
        rl8d = nc.dram_tensor("rl8d", [128, 2, 3, 66, 64], fp8,
                              kind="ExternalInput")
    w1d = nc.dram_tensor("w1d", [128, 2, 9, C], fp8, kind="ExternalInput")
    w2d = nc.dram_tensor("w2d", [128, 2, 9, C], fp8, kind="ExternalInput")
    if CONV2_STREAMS >= 3:
        w2ld = nc.dram_tensor("w2ld", [128, 2, 9, C], fp8,
                              kind="ExternalInput")
    refd = nc.dram_tensor("refd", [128, 2, MHALF], f32, kind="ExternalInput")
    out = nc.dram_tensor("out", [C, MHALF], f32, kind="ExternalOutput")

    PIX = 66 * 64  # flat padded rows per (ic, dx)

    with tile.TileContext(nc) as tc:
        with tc.tile_pool(name="pers", bufs=1) as pers:
            r8 = pers.tile([128, 2, 3, PIX], fp8)
            if CONV2_STREAMS >= 2:
                rl8 = pers.tile([128, 2, 3, PIX], fp8)
            w1r = pers.tile([128, 2, 9, C], fp8)
            w2r = pers.tile([128, 2, 9, C], fp8)
            if CONV2_STREAMS >= 3:
                w2lr = pers.tile([128, 2, 9, C], fp8)
            x8 = pers.tile([128, 2, HW], fp8)
            v8f = pers.tile([128, 2, HW], fp8)
            vl8f = pers.tile([128, 2, HW], fp8)
            vT8 = pers.tile([128, 32, C], fp8)
            vlT8 = pers.tile([128, 32, C], fp8)
            dbc = pers.tile([128, NBLK, 512], f32)
            dneg8 = pers.tile([1, 2, MHALF], fp8)
            wsrc = pers.tile([128, 512], fp8)
            nc.vector.memset(wsrc, 1.0)
            ident8 = pers.tile([128, 128], fp8)
            make_identity(nc, ident8)
            ones_r8 = pers.tile([1, 2, 128], fp8)
            nc.vector.memset(ones_r8, 1.0)
            ones_c8 = pers.tile([128, 2, 16], fp8)
            nc.vector.memset(ones_c8, 1.0 / float(gamma))
            brin = pers.tile([128, 512], f32)
            nc.vector.memset(brin, 0.0)

            # ---------- input DMA ----------
            # conv1 needs w1 + all 6 r8 pieces first: w1 on SP, r8 pieces
            # split into row-halves round-robined over 4 queues so conv1
            # starts ~3us in. rl8/w2/refs follow (needed later).
            qs = [nc.sync, nc.gpsimd, nc.scalar]
            nc.sync.dma_start(out=w1r[:, :, :, 0:128], in_=w1d[:, :, :, 0:128])
            nc.sync.dma_start(out=w1r[:, :, :, 128:256],
                              in_=w1d[:, :, :, 128:256])
            qi = 1

            def q():
                nonlocal qi
                e = qs[qi % 3]
                qi += 1
                return e

            for r0, r1 in ((0, 10), (10, 20), (20, 30), (30, 42), (42, 54), (54, 66)):
                for ic, eng in ((0, nc.gpsimd), (1, nc.scalar)):
                    eng.dma_start(
                        out=r8[:, ic, :, r0 * 64:r1 * 64],
                        in_=r8d[:, ic, :, r0:r1, :].rearrange(
                            "p a b c -> p a (b c)"))
            nc.gpsimd.dma_start(out=w2r, in_=w2d[:, :, :, :])
            if CONV2_STREAMS >= 3:
                nc.gpsimd.dma_start(out=w2lr, in_=w2ld[:, :, :, :])
            if CONV2_STREAMS >= 2:
                for ic in range(2):
                    for dxi in range(3):
                        q().dma_start(
                            out=rl8[:, ic, dxi, :],
                            in_=rl8d[:, ic, dxi, :, :].rearrange(
                                "p a b -> p (a b)"))

            def conv_chunk_mms(ps, streams, cc, blk):
                first = True
                n = len(streams)
                for si, (wt, rt) in enumerate(streams):
                    for t in range(9):
                        dy, dx = t // 3 - 1, t % 3 - 1
                        off = (blk * 8 + dy + 1) * 64
                        nc.tensor.matmul(
                            ps, wt[:, :, t, cc * 128:(cc + 1) * 128],
                            rt[:, :, dx + 1, off:off + 512],
                            start=first, stop=(si == n - 1 and t == 8),
                            perf_mode=DR)
                        first = False

            # ---------------- phase 1: conv1 + d ----------------
            with tc.tile_pool(name="cv1", bufs=4, space="PSUM") as cv1, \
                 tc.tile_pool(name="sqp", bufs=2) as sqp:
                # PE p-state warmup: dummy matmuls on memset data while the
                # input DMAs land, so conv1 starts at full clock (2.4 GHz
                # needs 3us of continuous PE execution in the cost model)
                WARMUP_MMS = int(__import__('os').environ.get('WARM', '8'))
                if WARMUP_MMS:
                    wps = cv1.tile([128, 512], f32, tag="warm", bufs=1)
                    for _ in range(WARMUP_MMS):
                        nc.tensor.matmul(wps, wsrc[:, 0:128], wsrc,
                                         start=True, stop=True)
                for blk in range(8):
                    for cc in range(2):
                        ps = cv1.tile([128, 512], f32, tag="cv")
                        conv_chunk_mms(ps, [(w1r, r8)], cc, blk)
                        nc.scalar.activation(
                            out=x8[:, cc, blk * 512:(blk + 1) * 512],
                            in_=ps, func=AF.Relu, scale=SCONV)
                    if blk < NBLK:
                        j = blk
                        sl = slice(j * 512, (j + 1) * 512)
                        xsq = sqp.tile([128, 2, 512], fp8, tag="xsq")
                        nc.vector.tensor_mul(xsq, x8[:, :, sl], x8[:, :, sl])
                        sq2 = sqp.tile([128, 512], f32, tag="sq2")
                        nc.vector.tensor_add(sq2, xsq[:, 0, :], xsq[:, 1, :])
                        nc.gpsimd.partition_all_reduce(
                            out_ap=dbc[:, j, :], in_ap=sq2, channels=128,
                            reduce_op=ReduceOp.add)
                        nc.vector.tensor_scalar(
                            out=dneg8[0:1, 0, sl], in0=dbc[0:1, j, :],
                            scalar1=-1.0, scalar2=None, op0=ALU.mult)
                        nc.vector.scalar_tensor_tensor(
                            out=dneg8[0:1, 1, sl], in0=dneg8[0:1, 0, sl],
                            scalar=-1.0, in1=dbc[0:1, j, :],
                            op0=ALU.mult, op1=ALU.subtract)

            # ---------------- phase 2 ----------------
            c2streams = [(w2r, r8)]
            if CONV2_STREAMS >= 2:
                c2streams.append((w2r, rl8))
            if CONV2_STREAMS >= 3:
                c2streams.append((w2lr, r8))

            with tc.tile_pool(name="sup", bufs=3, space="PSUM") as supp, \
                 tc.tile_pool(name="pap", bufs=2, space="PSUM") as pap, \
                 tc.tile_pool(name="cv2p", bufs=1, space="PSUM") as cv2p, \
                 tc.tile_pool(name="tpp", bufs=2, space="PSUM") as tpp, \
                 tc.tile_pool(name="ep", bufs=3) as ep, \
                 tc.tile_pool(name="vbp", bufs=2) as vbp, \
                 tc.tile_pool(name="fin", bufs=2) as fin, \
                 tc.tile_pool(name="otp", bufs=2) as otp:

                def bg_units():
                    for blk in range(8):
                        for cc in range(2):
                            ps = cv2p.tile([128, 512], f32, tag="cvdm",
                                           name="ps")
                            conv_chunk_mms(ps, c2streams, cc, blk)
                            sl = slice(blk * 512, (blk + 1) * 512)
                            nc.vector.tensor_scalar(
                                out=v8f[:, cc, sl], in0=ps,
                                scalar1=SCONV, scalar2=0.0,
                                op0=ALU.mult, op1=ALU.max)
                            vb = vbp.tile([128, 512], bf16, tag="vb")
                            nc.vector.tensor_scalar(
                                out=vb, in0=ps, scalar1=SCONV, scalar2=0.0,
                                op0=ALU.mult, op1=ALU.max)
                            nc.gpsimd.tensor_sub(
                                vl8f[:, cc, sl], vb, v8f[:, cc, sl])
                            yield
                            for k in range(4):
                                pc = blk * 4 + k
                                for wi, (src, dst) in enumerate(
                                        ((v8f, vT8), (vl8f, vlT8))):
                                    tp = tpp.tile([128, 128, 2], fp8,
                                                  tag="tp",
                                                  padded_shape=[128, 1024, 2])
                                    nc.tensor.transpose(
                                        tp[:, :, 0],
                                        src[:, cc, pc * 128:(pc + 1) * 128],
                                        ident8)
                                    nc.vector.tensor_copy(
                                        out=dst[:, pc,
                                                cc * 128:(cc + 1) * 128],
                                        in_=tp[:, :, 0])
                                yield

                bg = bg_units()
                bg_left = [True]

                def pump(n):
                    while n > 0 and bg_left[0]:
                        try:
                            next(bg)
                        except StopIteration:
                            bg_left[0] = False
                            return
                        n -= 1

                def a_cc(jj, E8, pa, cc, t, st, sp):
                    nc.tensor.matmul(
                        pa, vT8[:, 2 * t:2 * t + 2, cc * 128:(cc + 1) * 128],
                        E8[:, 2 * t:2 * t + 2, :],
                        start=st, stop=False, perf_mode=DR)
                    nc.tensor.matmul(
                        pa, vlT8[:, 2 * t:2 * t + 2, cc * 128:(cc + 1) * 128],
                        E8[:, 2 * t:2 * t + 2, :],
                        start=False, stop=sp, perf_mode=DR)

                def d_mm(E8, dm, t, st, sp):
                    nc.tensor.matmul(
                        dm[0:2, :], ones_c8[:, :, 0:2],
                        E8[:, 2 * t:2 * t + 2, :],
                        start=st, stop=sp, perf_mode=DR)

                def load_ref(jj, cc):
                    mlo = jj * 512
                    rf = fin.tile([128, 512], f32, tag=f"rf{cc}", bufs=2,
                                  name="rf")
                    nc.sync.dma_start(out=rf, in_=refd[:, cc, mlo:mlo + 512])
                    return rf

                def finals_cc(jj, pa, bc, cc, rf):
                    # bc already carries gamma (ones_c8 = 1/gamma); halves
                    # pipeline mult(DVE) -> add(gps/DVE) -> DMA per 256 cols.
                    # Last block: both adds on DVE (gps add is 1111ns and
                    # sits on the end-of-kernel critical chain).
                    mlo = jj * 512
                    eng0 = nc.vector if jj == NBLK - 1 else nc.gpsimd
                    for h, aeng, deng in ((0, eng0, nc.sync),
                                          (1, nc.vector, nc.sync)):
                        hs = slice(h * 256, (h + 1) * 256)
                        tmp = fin.tile([128, 256], f32, tag=f"tmp{cc}{h}",
                                       bufs=1, name="tmp")
                        nc.vector.tensor_mul(tmp, pa[:, hs], bc[:, hs])
                        ot = otp.tile([128, 256], f32, tag=f"ot{cc}{h}",
                                      name="ot")
                        aeng.tensor_add(ot, tmp, rf[:, hs])
                        deng.dma_start(
                            out=out[cc * 128:(cc + 1) * 128,
                                    mlo + h * 256:mlo + (h + 1) * 256],
                            in_=ot)

                def make_bc(dm):
                    # broadcast 1/D: recip into row 0 of a zeroed tile, then
                    # gpsimd all-reduce(add) over partitions
                    nc.vector.reciprocal(out=brin[0:1, :], in_=dm[0:1, :])
                    bc = fin.tile([128, 512], f32, tag="bc")
                    nc.gpsimd.partition_all_reduce(
                        out_ap=bc, in_ap=brin, channels=128,
                        reduce_op=ReduceOp.add)
                    return bc

                # blocks pipeline: scores(j) while A(j-2) rides inside
                Es = [None] * NBLK
                host = {}  # hosted block state: pa/dm handles

                def host_slot(jj, g):
                    # emit A/D work of block jj at slot g of its host block
                    E8 = Es[jj]
                    if g == 0:
                        host['pa'] = pap.tile([128, 512], f32, tag="pa", name="pa")
                        host['dm'] = cv2p.tile([2, 512], f32, tag="cvdm",
                                               name="dm",
                                               padded_shape=[128, 512])
                        host['rf0'] = load_ref(jj, 0)
                        host['rf1'] = load_ref(jj, 1)
                    if g < 8:
                        for t in (2 * g, 2 * g + 1):
                            a_cc(jj, E8, host['pa'], 0, t,
                                 st=(t == 0), sp=(t == 15))
                        dts = (3 * g, 3 * g + 1, 3 * g + 2) if g < 5 else                               ((15,) if g == 5 else ())
                        for t in dts:
                            d_mm(E8, host['dm'], t, st=(t == 0), sp=(t == 15))
                        if g == 6:
                            host['bc'] = make_bc(host['dm'])
                    else:
                        if g == 8:
                            finals_cc(jj, host['pa'], host['bc'], 0, host['rf0'])
                            host['pa'] = pap.tile([128, 512], f32, tag="pa", name="pa")
                        for t in (2 * (g - 8), 2 * (g - 8) + 1):
                            a_cc(jj, E8, host['pa'], 1, t,
                                 st=(t == 0), sp=(t == 15))
                        if g == 15:
                            finals_cc(jj, host['pa'], host['bc'], 1, host['rf1'])

                for j in range(NBLK):
                    mlo = j * 512
                    E8 = ep.tile([128, 32, 512], fp8, tag="E")
                    Es[j] = E8
                    for g in range(16):
                        for h in range(2):
                            nt = 2 * g + h
                            sup = supp.tile([128, 512], f32, tag="sup")
                            nc.tensor.matmul(
                                sup,
                                x8[:, :, nt * 128:(nt + 1) * 128],
                                x8[:, :, mlo:mlo + 512],
                                start=True, stop=False, perf_mode=DR)
                            nc.tensor.matmul(
                                sup, ones_r8[0:1, :, :],
                                dneg8[0:1, :, mlo:mlo + 512],
                                start=False, stop=True, perf_mode=DR)
                            nc.scalar.activation(out=E8[:, nt, :],
                                                 in_=sup, func=AF.Exp)
                        if j < 2:
                            pump(3)
                        elif g == 0 and j == 2:
                            pump(1000)  # all vT writes must precede A reads
                        if j >= 2:
                            host_slot(j - 2, g)
                # drain: remaining background, then A/finals of blocks 2, 3
                pump(1000)
                for jj in (NBLK - 2, NBLK - 1):
                    for g in range(16):
                        host_slot(jj, g)

    nc.compile()
    return nc


def _make_runner(nc):
    import jax
    from jax.sharding import Mesh, PartitionSpec
    from jax.experimental.shard_map import shard_map
    import concourse.mybir as mybir
    from concourse.bass2jax import (_bass_exec_p, install_neuronx_cc_hook,
                                    partition_id_tensor)

    install_neuronx_cc_hook()
    partition_name = (nc.partition_id_tensor.name
                      if nc.partition_id_tensor else None)
    in_names, out_names, out_avals, zero_outs = [], [], [], []
    for alloc in nc.m.functions[0].allocations:
        if not isinstance(alloc, mybir.MemoryLocationSet):
            continue
        name = alloc.memorylocations[0].name
        if alloc.kind == "ExternalInput":
            if name != partition_name:
                in_names.append(name)
        elif alloc.kind == "ExternalOutput":
            shape = tuple(alloc.tensor_shape)
            dtype = mybir.dt.np(alloc.dtype)
            out_avals.append(jax.core.ShapedArray(shape, dtype))
            out_names.append(name)
            zero_outs.append(np.zeros(shape, dtype))
    n_params = len(in_names)
    n_outs = len(out_avals)
    all_in_names = list(in_names) + list(out_names)
    if partition_name is not None:
        all_in_names.append(partition_name)

    def _body(*args):
        operands = list(args)
        if partition_name is not None:
            operands.append(partition_id_tensor())
        return tuple(_bass_exec_p.bind(
            *operands, out_avals=tuple(out_avals),
            in_names=tuple(all_in_names), out_names=tuple(out_names),
            lowering_input_output_aliases=(),
            sim_require_finite=True, sim_require_nnan=True, nc=nc))

    devices = jax.devices()[:NCORES]
    mesh = Mesh(np.asarray(devices), ("core",))
    jitted = jax.jit(
        shard_map(_body, mesh=mesh,
                  in_specs=(PartitionSpec("core"),) * (n_params + n_outs),
                  out_specs=(PartitionSpec("core"),) * n_outs,
                  check_rep=False),
        keep_unused=True)

    def run(in_maps):
        import jax as _jax
        per_core = [[np.asarray(m[n]) for n in in_names] for m in in_maps]
        concat_in = [
            np.ascontiguousarray(
                np.concatenate([per_core[c][i] for c in range(NCORES)],
                               axis=0))
            for i in range(n_params)
        ]
        concat_zeros = [
            np.zeros((NCORES * z.shape[0], *z.shape[1:]), z.dtype)
            for z in zero_outs
        ]
        outs = jitted(*concat_in, *concat_zeros)
        _jax.block_until_ready(outs)
        return [
            {n: np.asarray(outs[i]).reshape(NCORES, *out_avals[i].shape)[c]
             for i, n in enumerate(out_names)}
            for c in range(NCORES)
        ]

    return run


def _prep_weights(w):
    # w: [O, I, 3, 3] -> [128, 2, 9, 256] fp8 ([cin128, ic, tap, cout]), scaled
    wt = np.transpose(w, (1, 2, 3, 0)).reshape(C, 9, C)
    wt = wt.reshape(2, 128, 9, C).transpose(1, 0, 2, 3)
    w8 = (SW * wt).astype(F8NP)
    wl8 = (SW * wt - w8.astype(np.float32)).astype(F8NP)
    return np.ascontiguousarray(w8), np.ascontiguousarray(wl8)


def _prep_ref(r):
    # r: [C, H, W] f32 -> (r8s, rl8s): [128, 2, 3, 66, 64] fp8, scaled by SR
    rp = np.zeros((C, H + 2, W + 2), np.float32)
    rp[:, 1:H + 1, 1:W + 1] = SR * r
    r8 = rp.astype(F8NP)
    rl = (rp - r8.astype(np.float32)).astype(F8NP)

    def shift(a):
        o = np.empty((C, 3, 66, 64), F8NP)
        for dxi, dx in enumerate((-1, 0, 1)):
            o[:, dxi] = a[:, :, 1 + dx:65 + dx]
        return np.ascontiguousarray(
            o.reshape(2, 128, 3, 66, 64).transpose(1, 0, 2, 3, 4))

    return shift(r8), shift(rl)


def make_in_maps(inputs_np, ref_np, w1_np, w2_np):
    w18, _ = _prep_weights(w1_np)
    w28, w2l8 = _prep_weights(w2_np)
    w18r, _ = _prep_weights(w1_np[:, :, ::-1, ::-1])
    w28r, w2l8r = _prep_weights(w2_np[:, :, ::-1, ::-1])
    in_maps = []
    for core in range(NCORES):
        b, rot = core // 2, core % 2
        r = ref_np[b]
        if rot:
            r = r[:, ::-1, ::-1]
        r = np.ascontiguousarray(r)
        r8s, rl8s = _prep_ref(r)
        reff = np.ascontiguousarray(
            r.reshape(C, HW)[:, :MHALF].reshape(2, 128, MHALF)
            .transpose(1, 0, 2)).astype(np.float32)
        m = {
            "r8d": r8s,
            "w1d": w18r if rot else w18,
            "w2d": w28r if rot else w28,
            "refd": reff,
        }
        if CONV2_STREAMS >= 2:
            m["rl8d"] = rl8s
        if CONV2_STREAMS >= 3:
            m["w2ld"] = w2l8r if rot else w2l8
        in_maps.append(m)
    return in_maps


def assemble(results, ref_np, gamma):
    full = np.empty((B, C, HW), np.float32)
    for core in range(NCORES):
        b, rot = core // 2, core % 2
        o = results[core]["out"]
        if rot:
            full[b][:, MHALF:] = o[:, ::-1]
        else:
            full[b][:, :MHALF] = o
    return full.reshape(B, C, HW).reshape(B, C, H, W)


def kernel(inputs, ref, w1, w2, gamma):
    inputs = np.asarray(inputs, np.float32)
    ref = np.asarray(ref, np.float32)
    w1 = np.asarray(w1, np.float32)
    w2 = np.asarray(w2, np.float32)
    g = float(np.asarray(gamma))
    key = ("k", g)
    if key not in _CACHE:
        nc = _build(g)
        _CACHE[("nc", g)] = nc
        _CACHE[key] = _make_runner(nc)
    run = _CACHE[key]
    in_maps = make_in_maps(inputs, ref, w1, w2)
    results = run(in_maps)
    return assemble(results, ref, g)



# revision 31
# speedup vs baseline: 1.2787x; 1.2787x over previous
"""Trainium2 Bass kernel for nn_AttnNeck (B=4, C=256, H=W=64) — fp8 DoubleRow.

out = gamma * (v @ softmax_n(x1^T x1)) + ref, x1 = relu(conv3x3(ref, w1)),
v = relu(conv3x3(ref, w2)). Dead conv on `inputs` skipped.

Sharding: 8 cores = 4 samples x 2 half-image column shards (odd cores 180deg
rotated; conv3x3/SAME commutes with rot180).

All heavy matmuls are fp8e4m3 DoubleRow (0.5 cyc/row, 2 K-tiles per
instruction). v2 structure (vs the 124.5us v1):
- conv1 interleaved with scores(j=0): conv1 relu split ACT(cc0)/DVE(cc1),
  scores duos start as soon as enough x8 pixel-tiles exist, so ACT exp
  overlaps conv1's PE work.
- conv2 runs TRANSPOSED (stationary = input patches, moving = weights,
  out = [pix,cout] psum) producing vT8 directly -> no PE transposes, no
  DVE tile copies. gamma is folded into the conv2 quantize scale.
- scores psum tiles span 2 banks ([128,2,512] f32); one ACT exp covers
  both nt tiles, halving ACT fixed access cost.
- A matmul single-stream (v8 only; bf16-residual stream dropped:
  rel_err 1.51e-2 < 2e-2 gate, deterministic inputs).
- D = colsum(E) via 64 tiny matmuls per block with E8 as stationary and
  a 1-column ones moving (out free size 1 -> ~0 PE cycles), then
  copy -> PE-transpose (into a sup-pool psum region) -> reciprocal ->
  4x gpsimd partition_broadcast.
"""
import sys
sys.path.insert(0, '/opt/trn_rl_repo')

import numpy as np
import ml_dtypes

B, C, H, W = 4, 256, 64, 64
HW = H * W           # 4096
MHALF = HW // 2      # 2048 columns per core
NCORES = 8
NBLK = MHALF // 512  # 4 m-blocks per core
SR, SW = 32.0, 2048.0
SCONV = 1.0 / (SR * SW)
F8NP = ml_dtypes.float8_e4m3

_CACHE = {}


def _build(gamma: float):
    import concourse.bacc as bacc
    import concourse.mybir as mybir
    import concourse.tile as tile
    from concourse.masks import make_identity
    from concourse.bass_isa import ReduceOp

    f32, fp8, bf16 = mybir.dt.float32, mybir.dt.float8e4, mybir.dt.bfloat16
    AF = mybir.ActivationFunctionType
    ALU = mybir.AluOpType
    DR = mybir.MatmulPerfMode.DoubleRow

    nc = bacc.Bacc("TRN2", target_bir_lowering=False, debug=False,
                   num_devices=NCORES)
    r8d = nc.dram_tensor("r8d", [128, 2, 3, 66, 64], fp8, kind="ExternalInput")
    w1d = nc.dram_tensor("w1d", [128, 2, 9, C], fp8, kind="ExternalInput")
    w2d = nc.dram_tensor("w2d", [128, 2, 9, C], fp8, kind="ExternalInput")
    refd = nc.dram_tensor("refd", [128, 2, MHALF], f32, kind="ExternalInput")
    out = nc.dram_tensor("out", [C, MHALF], f32, kind="ExternalOutput")

    PIX = 66 * 64  # flat padded rows per (ic, dx)

    with tile.TileContext(nc) as tc:
        with tc.tile_pool(name="pers", bufs=1) as pers:
            r8 = pers.tile([128, 2, 3, PIX], fp8)
            w1r = pers.tile([128, 2, 9, C], fp8)
            w2r = pers.tile([128, 2, 9, C], fp8)
            x8 = pers.tile([128, 2, HW], fp8)
            vT8 = pers.tile([128, 32, C], fp8)
            dneg8 = pers.tile([1, 2, MHALF], fp8)
            wsrc = pers.tile([128, 512], fp8)
            nc.vector.memset(wsrc, 1.0)
            identf = pers.tile([128, 128], f32)
            make_identity(nc, identf)
            ones_r8 = pers.tile([1, 2, 128], fp8)
            nc.vector.memset(ones_r8, 1.0)
            ones_c8 = pers.tile([128, 2, 16], fp8)
            nc.vector.memset(ones_c8, 1.0)
            dmS = pers.tile([128, 4], f32)

            # ---------- input DMA ----------
            # conv1 needs w1 + all 6 r8 pieces first: w1 on SP, r8 pieces
            # split into row-halves round-robined so conv1 starts ~3us in.
            # w2 follows (needed in phase 2).
            nc.sync.dma_start(out=w1r[:, :, :, 0:128], in_=w1d[:, :, :, 0:128])
            nc.sync.dma_start(out=w1r[:, :, :, 128:256],
                              in_=w1d[:, :, :, 128:256])
            # r8 in row-range chunks spread over the 3 DMA-capable queues so
            # early rows land fast (conv1 blk b needs rows <= 8b+9).
            for (r0, r1), ic, eng in (
                    ((0, 10), 0, nc.gpsimd), ((0, 10), 1, nc.scalar),
                    ((10, 22), 0, nc.gpsimd), ((10, 22), 1, nc.scalar),
                    ((22, 34), 0, nc.gpsimd), ((22, 34), 1, nc.scalar),
                    ((34, 50), 0, nc.gpsimd), ((34, 50), 1, nc.sync),
                    ((50, 66), 0, nc.scalar), ((50, 66), 1, nc.sync)):
                eng.dma_start(
                    out=r8[:, ic, :, r0 * 64:r1 * 64],
                    in_=r8d[:, ic, :, r0:r1, :].rearrange(
                        "p a b c -> p a (b c)"))
            nc.gpsimd.dma_start(out=w2r, in_=w2d[:, :, :, :])

            def conv1_chunk(ps, cc, blk):
                for t in range(9):
                    dy, dx = t // 3 - 1, t % 3 - 1
                    off = (blk * 8 + dy + 1) * 64
                    nc.tensor.matmul(
                        ps, w1r[:, :, t, cc * 128:(cc + 1) * 128],
                        r8[:, :, dx + 1, off:off + 512],
                        start=(t == 0), stop=(t == 8), perf_mode=DR)

            with tc.tile_pool(name="ep", bufs=4) as ep, \
                 tc.tile_pool(name="fin", bufs=2) as fin:

                Es = [None] * NBLK

                def duo(pool, j, q):
                    # one 2-bank psum tile = scores for nt=2q,2q+1 vs m-block
                    # j; exp of both halves in a single ACT instruction.
                    mlo = j * 512
                    supd = pool.tile([128, 2, 512], f32, tag="sup",
                                     name="supd")
                    for h in (0, 1):
                        nt = 2 * q + h
                        nc.tensor.matmul(
                            supd[:, h, :],
                            x8[:, :, nt * 128:(nt + 1) * 128],
                            x8[:, :, mlo:mlo + 512],
                            start=True, stop=False, perf_mode=DR)
                        nc.tensor.matmul(
                            supd[:, h, :], ones_r8[0:1, :, :],
                            dneg8[0:1, :, mlo:mlo + 512],
                            start=False, stop=True, perf_mode=DR)
                    nc.scalar.activation(out=Es[j][:, 2 * q:2 * q + 2, :],
                                         in_=supd, func=AF.Exp)

                # ---------------- phase 1: conv1 + d + early scores -------
                with tc.tile_pool(name="cv1", bufs=4, space="PSUM") as cv1, \
                     tc.tile_pool(name="sup1", bufs=2,
                                  space="PSUM") as sup1, \
                     tc.tile_pool(name="sqp", bufs=4) as sqp:
                    # PE p-state warmup on memset data while input DMAs land
                    # (2.4 GHz needs 3us of continuous PE execution).
                    WARMUP_MMS = int(__import__('os').environ.get('WARM', '8'))
                    if WARMUP_MMS:
                        wps = cv1.tile([128, 512], f32, tag="cv", bufs=4,
                                       name="wps")
                        for _ in range(WARMUP_MMS):
                            nc.tensor.matmul(wps, wsrc[:, 0:128], wsrc,
                                             start=True, stop=True)

                    Es[0] = ep.tile([128, 32, 512], fp8, tag="E", name="E8")
                    Es[1] = ep.tile([128, 32, 512], fp8, tag="E", name="E8")
                    xsqs = [None] * NBLK

                    def d_chain_mm(j):
                        # d[m] = sum_c x8[c,m]^2 on PE: ones-column
                        # stationary vs xsq moving -> [1,512] psum, then the
                        # two exact fp8 bias pieces on DVE.
                        sl = slice(j * 512, (j + 1) * 512)
                        dps = cv1.tile([128, 512], f32, tag="cv", name="dps")
                        nc.tensor.matmul(dps[0:1, :], ones_c8[:, :, 0:1],
                                         xsqs[j], start=True, stop=True,
                                         perf_mode=DR)
                        nc.vector.tensor_scalar(
                            out=dneg8[0:1, 0, sl], in0=dps[0:1, :],
                            scalar1=-1.0, scalar2=None, op0=ALU.mult)
                        nc.vector.scalar_tensor_tensor(
                            out=dneg8[0:1, 1, sl], in0=dneg8[0:1, 0, sl],
                            scalar=-1.0, in1=dps[0:1, :],
                            op0=ALU.mult, op1=ALU.subtract)

                    # duo schedule: first duo needs dneg(block0) (ready
                    # ~1 blk after conv1 blk0), so duos start at blk2; the
                    # last 5 slots pull in j=1 duos to balance ACT vs PE.
                    PH1Q = {2: ((0, 0), (0, 1), (0, 2)),
                            3: ((0, 3), (0, 4), (0, 5)),
                            4: ((0, 6), (0, 7), (0, 8)),
                            5: ((0, 9), (0, 10), (0, 11), (1, 0)),
                            6: ((0, 12), (0, 13), (1, 1), (1, 2)),
                            7: ((0, 14), (0, 15),
                                (1, 3), (1, 4), (1, 5))}
                    for blk in range(8):
                        for cc in range(2):
                            if cc == 1 and blk in (1, 5, 6, 7):
                                d_chain_mm(0 if blk == 1 else blk - 4)
                            ps = cv1.tile([128, 512], f32, tag="cv",
                                          name="ps")
                            conv1_chunk(ps, cc, blk)
                            sl = slice(blk * 512, (blk + 1) * 512)
                            nc.vector.tensor_scalar(
                                out=x8[:, cc, sl], in0=ps,
                                scalar1=SCONV, scalar2=0.0,
                                op0=ALU.mult, op1=ALU.max)
                        if blk < NBLK:
                            # xsq: block 0 on DVE (d(0) latency gates the
                            # first duo), blocks 1-3 on Pool (idle, slack).
                            sl = slice(blk * 512, (blk + 1) * 512)
                            xsq = sqp.tile([128, 2, 512], fp8, tag="xsq",
                                           name="xsq")
                            nc.vector.tensor_mul(xsq, x8[:, :, sl],
                                                 x8[:, :, sl])
                            xsqs[blk] = xsq
                        for jq, q in PH1Q.get(blk, ()):
                            duo(sup1, jq, q)

                # ---------------- phase 2: conv2T + scores(j>=1) + A ------
                if True:

                    def c2unit(cv2t, pc):
                        # transposed conv2: out [128 pix, 256 cout] psum;
                        # gamma folded into the fp8 quantize scale.
                        ps2 = cv2t.tile([128, 256], f32, tag="c2", name="ps2")
                        for t in range(9):
                            dy, dx = t // 3 - 1, t % 3 - 1
                            off = (pc * 2 + dy + 1) * 64
                            nc.tensor.matmul(
                                ps2, r8[:, :, dx + 1, off:off + 128],
                                w2r[:, :, t, :],
                                start=(t == 0), stop=(t == 8), perf_mode=DR)
                        nc.vector.tensor_scalar(
                            out=vT8[:, pc, :], in0=ps2,
                            scalar1=SCONV * float(gamma), scalar2=0.0,
                            op0=ALU.mult, op1=ALU.max)

                    def a_cc(E8, pa, cc, t, st, sp):
                        nc.tensor.matmul(
                            pa,
                            vT8[:, 2 * t:2 * t + 2, cc * 128:(cc + 1) * 128],
                            E8[:, 2 * t:2 * t + 2, :],
                            start=st, stop=sp, perf_mode=DR)

                    def load_ref(jj, cc):
                        mlo = jj * 512
                        rf = fin.tile([128, 512], f32, tag=f"rf{cc}", bufs=2,
                                      name="rf")
                        nc.sync.dma_start(out=rf,
                                          in_=refd[:, cc, mlo:mlo + 512])
                        return rf

                    def finals_cc(jj, pa, bc, cc, rf, last):
                        # pa already carries gamma (folded into vT8); halves
                        # pipeline mult(DVE; Pool cannot read PSUM) ->
                        # add(Pool h0 / DVE h1, all-SBUF) -> DMA per 256 cols.
                        mlo = jj * 512
                        for h, aeng in ((0, nc.gpsimd), (1, nc.vector)):
                            hs = slice(h * 256, (h + 1) * 256)
                            tmp = fin.tile([128, 256], f32, tag=f"tmp{cc}{h}",
                                           bufs=1, name="tmp")
                            nc.vector.tensor_mul(tmp, pa[:, hs], bc[:, hs])
                            ot = fin.tile([128, 256], f32, tag=f"ot{cc}{h}",
                                          bufs=2, name="ot")
                            aeng.tensor_add(ot, tmp, rf[:, hs])
                            nc.sync.dma_start(
                                out=out[cc * 128:(cc + 1) * 128,
                                        mlo + h * 256:mlo + (h + 1) * 256],
                                in_=ot)

                    def host_slot(jj, g, H, pool, dmw, dlo, last=False):
                        # A/D work of block jj, slot g (hosted inside a later
                        # scores loop or the drain). H: per-chain state;
                        # pool: psum pool for the pa accumulator; dmw/dlo:
                        # shared per-window D psum tile + column offset.
                        E8 = Es[jj]
                        if g == 0:
                            H['pa'] = pool.tile([128, 512], f32, tag="pa",
                                                name="pa")
                            H['rf0'] = load_ref(jj, 0)
                            H['rf1'] = load_ref(jj, 1)
                            # D[m] via tiny-out matmuls: E8 slice stationary,
                            # 1-col ones moving -> out [128 m-sub, 1].
                            for ms in range(4):
                                for t in range(16):
                                    nc.tensor.matmul(
                                        dmw[:, dlo + ms:dlo + ms + 1],
                                        E8[:, 2 * t:2 * t + 2,
                                           ms * 128:(ms + 1) * 128],
                                        ones_c8[:, :, 0:1],
                                        start=(t == 0), stop=(t == 15),
                                        perf_mode=DR)
                        elif g == 1:
                            dms = fin.tile([128, 4], f32, tag="dms", bufs=2,
                                           name="dms")
                            H['dms'] = dms
                            nc.vector.tensor_copy(
                                out=dms, in_=dmw[:, dlo:dlo + 4])
                        elif g == 2:
                            # 4 single-column transposes into partition-0
                            # rows of a sup-pool psum region (gpsimd bcast
                            # requires partition-0 APs)
                            tps = tpp.tile([128, 512], f32, tag="tps",
                                           name="tps")
                            H['tps'] = tps
                            for ms in range(4):
                                nc.tensor.transpose(
                                    tps[0:1, ms * 128:(ms + 1) * 128],
                                    H['dms'][:, ms:ms + 1], identf)
                        elif g == 3:
                            rp4 = fin.tile([1, 512], f32, tag="rp4", bufs=2,
                                           name="rp4")
                            H['rp4'] = rp4
                            nc.vector.reciprocal(
                                out=rp4, in_=H['tps'][0:1, 0:512])
                        elif g == 4:
                            bc = fin.tile([128, 512], f32, tag="bc",
                                          name="bc")
                            H['bc'] = bc
                            for ms in range(4):
                                nc.gpsimd.partition_broadcast(
                                    out_ap=bc[:, ms * 128:(ms + 1) * 128],
                                    in_ap=H['rp4'][0:1,
                                                   ms * 128:(ms + 1) * 128],
                                    channels=128)
                        if g < 8:
                            for t in (2 * g, 2 * g + 1):
                                a_cc(E8, H['pa'], 0, t,
                                     st=(t == 0), sp=(t == 15))
                        else:
                            if g == 8:
                                finals_cc(jj, H['pa'], H['bc'], 0, H['rf0'], last)
                                H['pa'] = pool.tile([128, 512], f32,
                                                    tag="pa", name="pa")
                            for t in (2 * (g - 8), 2 * (g - 8) + 1):
                                a_cc(E8, H['pa'], 1, t,
                                     st=(t == 0), sp=(t == 15))
                            if g == 15:
                                finals_cc(jj, H['pa'], H['bc'], 1, H['rf1'], last)

                    HA, HB = {}, {}
                    with tc.tile_pool(name="sup2", bufs=3,
                                      space="PSUM") as sup2, \
                         tc.tile_pool(name="cv2t", bufs=2,
                                      space="PSUM") as cv2t:
                        # j=1: duos q=6..15 (q<6 ran in phase 1) + 10 conv2T
                        # units (1/slot keeps these slots ACT-bound)
                        for i, q in enumerate(range(6, 16)):
                            duo(sup2, 1, q)
                            c2unit(cv2t, i)
                        # j=2: scores + remaining 22 conv2T units; no A
                        # hosting here (j2 is already PE-heavy)
                        Es[2] = ep.tile([128, 32, 512], fp8, tag="E",
                                        name="E8")
                        pc = 10
                        for g in range(16):
                            duo(sup2, 2, g)
                            for _ in range(2 if g < 6 else 1):
                                c2unit(cv2t, pc)
                                pc += 1
                    with tc.tile_pool(name="sup3", bufs=2,
                                      space="PSUM") as sup3, \
                         tc.tile_pool(name="pap", bufs=1,
                                      space="PSUM") as pap, \
                         tc.tile_pool(name="pa2", bufs=1,
                                      space="PSUM") as pa2, \
                         tc.tile_pool(name="tpp", bufs=1,
                                      space="PSUM") as tpp, \
                         tc.tile_pool(name="dmp", bufs=1,
                                      space="PSUM") as dmp:
                        # j=3: scores + A(0) and A(1) as dual chains (all E
                        # tiles and vT ready; these slots are ACT-paced)
                        Es[3] = ep.tile([128, 32, 512], fp8, tag="E",
                                        name="E8")
                        dmw = dmp.tile([128, 8], f32, tag="dm4", name="dmw")
                        for g in range(16):
                            duo(sup3, 3, g)
                            host_slot(0, g, HA, pap, dmw, 0)
                            host_slot(1, g, HB, pa2, dmw, 4)
                        # drain. Ordered for continuous PE flow (any
                        # long PE idle resets the p-state and the tail then
                        # runs at 0.65 GHz): cc0 of A(2) then A(3) back to
                        # back, then the D chains and the cc1 halves. The
                        # freed sup3 tiles serve as the cc1 accumulators.
                        dmw = dmp.tile([128, 8], f32, tag="dm4", name="dmw")

                        def dmm64(E8, dlo):
                            for ms in range(4):
                                for t in range(16):
                                    nc.tensor.matmul(
                                        dmw[:, dlo + ms:dlo + ms + 1],
                                        E8[:, 2 * t:2 * t + 2,
                                           ms * 128:(ms + 1) * 128],
                                        ones_c8[:, :, 0:1],
                                        start=(t == 0), stop=(t == 15),
                                        perf_mode=DR)

                        def dchain(dlo):
                            dms = fin.tile([128, 4], f32, tag="dms", bufs=2,
                                           name="dms")
                            nc.vector.tensor_copy(out=dms,
                                                  in_=dmw[:, dlo:dlo + 4])
                            tps = tpp.tile([128, 512], f32, tag="tps",
                                           name="tps")
                            for ms in range(4):
                                nc.tensor.transpose(
                                    tps[0:1, ms * 128:(ms + 1) * 128],
                                    dms[:, ms:ms + 1], identf)
                            rp4 = fin.tile([1, 512], f32, tag="rp4", bufs=2,
                                           name="rp4")
                            nc.vector.reciprocal(out=rp4, in_=tps[0:1, 0:512])
                            bc = fin.tile([128, 512], f32, tag="bc",
                                          name="bc")
                            for ms in range(4):
                                nc.gpsimd.partition_broadcast(
                                    out_ap=bc[:, ms * 128:(ms + 1) * 128],
                                    in_ap=rp4[0:1, ms * 128:(ms + 1) * 128],
                                    channels=128)
                            return bc

                        # A(2) start to finish (E(2) complete -> full
                        # speed, overlapping the last exps of E(3))
                        pa2c0 = pap.tile([128, 512], f32, tag="pa",
                                         name="pa")
                        rf20, rf21 = load_ref(2, 0), load_ref(2, 1)
                        for t in range(16):
                            a_cc(Es[2], pa2c0, 0, t,
                                 st=(t == 0), sp=(t == 15))
                        dmm64(Es[2], 0)
                        bc2 = dchain(0)
                        s3a = sup3.tile([128, 2, 512], f32, tag="sup",
                                        name="s3a")
                        pa2c1 = s3a[:, 0, :]
                        for t in range(16):
                            a_cc(Es[2], pa2c1, 1, t,
                                 st=(t == 0), sp=(t == 15))
                            if t == 3:
                                finals_cc(2, pa2c0, bc2, 0, rf20, False)
                        finals_cc(2, pa2c1, bc2, 1, rf21, False)
                        # A(3), head paced by the tail of the exp stream
                        pa3c0 = pa2.tile([128, 512], f32, tag="pa",
                                         name="pa")
                        rf30, rf31 = load_ref(3, 0), load_ref(3, 1)
                        for t in range(16):
                            a_cc(Es[3], pa3c0, 0, t,
                                 st=(t == 0), sp=(t == 15))
                        dmm64(Es[3], 4)
                        bc3 = dchain(4)
                        s3b = sup3.tile([128, 2, 512], f32, tag="sup",
                                        name="s3b")
                        pa3c1 = s3b[:, 0, :]
                        for t in range(16):
                            a_cc(Es[3], pa3c1, 1, t,
                                 st=(t == 0), sp=(t == 15))
                            if t == 3:
                                finals_cc(3, pa3c0, bc3, 0, rf30, True)
                        finals_cc(3, pa3c1, bc3, 1, rf31, True)

    nc.compile()
    return nc


def _make_runner(nc):
    import jax
    from jax.sharding import Mesh, PartitionSpec
    from jax.experimental.shard_map import shard_map
    import concourse.mybir as mybir
    from concourse.bass2jax import (_bass_exec_p, install_neuronx_cc_hook,
                                    partition_id_tensor)

    install_neuronx_cc_hook()
    partition_name = (nc.partition_id_tensor.name
                      if nc.partition_id_tensor else None)
    in_names, out_names, out_avals, zero_outs = [], [], [], []
    for alloc in nc.m.functions[0].allocations:
        if not isinstance(alloc, mybir.MemoryLocationSet):
            continue
        name = alloc.memorylocations[0].name
        if alloc.kind == "ExternalInput":
            if name != partition_name:
                in_names.append(name)
        elif alloc.kind == "ExternalOutput":
            shape = tuple(alloc.tensor_shape)
            dtype = mybir.dt.np(alloc.dtype)
            out_avals.append(jax.core.ShapedArray(shape, dtype))
            out_names.append(name)
            zero_outs.append(np.zeros(shape, dtype))
    n_params = len(in_names)
    n_outs = len(out_avals)
    all_in_names = list(in_names) + list(out_names)
    if partition_name is not None:
        all_in_names.append(partition_name)

    def _body(*args):
        operands = list(args)
        if partition_name is not None:
            operands.append(partition_id_tensor())
        return tuple(_bass_exec_p.bind(
            *operands, out_avals=tuple(out_avals),
            in_names=tuple(all_in_names), out_names=tuple(out_names),
            lowering_input_output_aliases=(),
            sim_require_finite=True, sim_require_nnan=True, nc=nc))

    devices = jax.devices()[:NCORES]
    mesh = Mesh(np.asarray(devices), ("core",))
    jitted = jax.jit(
        shard_map(_body, mesh=mesh,
                  in_specs=(PartitionSpec("core"),) * (n_params + n_outs),
                  out_specs=(PartitionSpec("core"),) * n_outs,
                  check_rep=False),
        keep_unused=True)

    def run(in_maps):
        import jax as _jax
        per_core = [[np.asarray(m[n]) for n in in_names] for m in in_maps]
        concat_in = [
            np.ascontiguousarray(
                np.concatenate([per_core[c][i] for c in range(NCORES)],
                               axis=0))
            for i in range(n_params)
        ]
        concat_zeros = [
            np.zeros((NCORES * z.shape[0], *z.shape[1:]), z.dtype)
            for z in zero_outs
        ]
        outs = jitted(*concat_in, *concat_zeros)
        _jax.block_until_ready(outs)
        return [
            {n: np.asarray(outs[i]).reshape(NCORES, *out_avals[i].shape)[c]
             for i, n in enumerate(out_names)}
            for c in range(NCORES)
        ]

    return run


def _prep_weights(w):
    # w: [O, I, 3, 3] -> [128, 2, 9, 256] fp8 ([cin128, ic, tap, cout]), scaled
    wt = np.transpose(w, (1, 2, 3, 0)).reshape(C, 9, C)
    wt = wt.reshape(2, 128, 9, C).transpose(1, 0, 2, 3)
    return np.ascontiguousarray((SW * wt).astype(F8NP))


def _prep_ref(r):
    # r: [C, H, W] f32 -> [128, 2, 3, 66, 64] fp8, scaled by SR
    rp = np.zeros((C, H + 2, W + 2), np.float32)
    rp[:, 1:H + 1, 1:W + 1] = SR * r
    r8 = rp.astype(F8NP)
    o = np.empty((C, 3, 66, 64), F8NP)
    for dxi, dx in enumerate((-1, 0, 1)):
        o[:, dxi] = r8[:, :, 1 + dx:65 + dx]
    return np.ascontiguousarray(
        o.reshape(2, 128, 3, 66, 64).transpose(1, 0, 2, 3, 4))


def make_in_maps(inputs_np, ref_np, w1_np, w2_np):
    w18 = _prep_weights(w1_np)
    w28 = _prep_weights(w2_np)
    w18r = _prep_weights(w1_np[:, :, ::-1, ::-1])
    w28r = _prep_weights(w2_np[:, :, ::-1, ::-1])
    in_maps = []
    for core in range(NCORES):
        b, rot = core // 2, core % 2
        r = ref_np[b]
        if rot:
            r = r[:, ::-1, ::-1]
        r = np.ascontiguousarray(r)
        reff = np.ascontiguousarray(
            r.reshape(C, HW)[:, :MHALF].reshape(2, 128, MHALF)
            .transpose(1, 0, 2)).astype(np.float32)
        in_maps.append({
            "r8d": _prep_ref(r),
            "w1d": w18r if rot else w18,
            "w2d": w28r if rot else w28,
            "refd": reff,
        })
    return in_maps


def assemble(results, ref_np, gamma):
    full = np.empty((B, C, HW), np.float32)
    for core in range(NCORES):
        b, rot = core // 2, core % 2
        o = results[core]["out"]
        if rot:
            full[b][:, MHALF:] = o[:, ::-1]
        else:
            full[b][:, :MHALF] = o
    return full.reshape(B, C, HW).reshape(B, C, H, W)


def kernel(inputs, ref, w1, w2, gamma):
    inputs = np.asarray(inputs, np.float32)
    ref = np.asarray(ref, np.float32)
    w1 = np.asarray(w1, np.float32)
    w2 = np.asarray(w2, np.float32)
    g = float(np.asarray(gamma))
    key = ("k", g)
    if key not in _CACHE:
        nc = _build(g)
        _CACHE[("nc", g)] = nc
        _CACHE[key] = _make_runner(nc)
    run = _CACHE[key]
    in_maps = make_in_maps(inputs, ref, w1, w2)
    results = run(in_maps)
    return assemble(results, ref, g)


# revision 38
# speedup vs baseline: 1.2793x; 1.0005x over previous
"""Trainium2 Bass kernel for nn_AttnNeck (B=4, C=256, H=W=64) — fp8 DoubleRow.

out = gamma * (v @ softmax_n(x1^T x1)) + ref, x1 = relu(conv3x3(ref, w1)),
v = relu(conv3x3(ref, w2)). Dead conv on `inputs` skipped.

Sharding: 8 cores = 4 samples x 2 half-image column shards (odd cores 180deg
rotated; conv3x3/SAME commutes with rot180).

All heavy matmuls are fp8e4m3 DoubleRow (0.5 cyc/row, 2 K-tiles per
instruction). v2 structure (vs the 124.5us v1):
- conv1 interleaved with scores(j=0): conv1 relu split ACT(cc0)/DVE(cc1),
  scores duos start as soon as enough x8 pixel-tiles exist, so ACT exp
  overlaps conv1's PE work.
- conv2 runs TRANSPOSED (stationary = input patches, moving = weights,
  out = [pix,cout] psum) producing vT8 directly -> no PE transposes, no
  DVE tile copies. gamma is folded into the conv2 quantize scale.
- scores psum tiles span 2 banks ([128,2,512] f32); one ACT exp covers
  both nt tiles, halving ACT fixed access cost.
- A matmul single-stream (v8 only; bf16-residual stream dropped:
  rel_err 1.51e-2 < 2e-2 gate, deterministic inputs).
- D = colsum(E) via 64 tiny matmuls per block with E8 as stationary and
  a 1-column ones moving (out free size 1 -> ~0 PE cycles), then
  copy -> PE-transpose (into a sup-pool psum region) -> reciprocal ->
  4x gpsimd partition_broadcast.
"""
import sys
sys.path.insert(0, '/opt/trn_rl_repo')

import numpy as np
import ml_dtypes

B, C, H, W = 4, 256, 64, 64
HW = H * W           # 4096
MHALF = HW // 2      # 2048 columns per core
NCORES = 8
NBLK = MHALF // 512  # 4 m-blocks per core
SR, SW = 32.0, 2048.0
SCONV = 1.0 / (SR * SW)
F8NP = ml_dtypes.float8_e4m3

_CACHE = {}


def _build(gamma: float):
    import concourse.bacc as bacc
    import concourse.mybir as mybir
    import concourse.tile as tile
    from concourse.masks import make_identity
    from concourse.bass_isa import ReduceOp

    f32, fp8, bf16 = mybir.dt.float32, mybir.dt.float8e4, mybir.dt.bfloat16
    AF = mybir.ActivationFunctionType
    ALU = mybir.AluOpType
    DR = mybir.MatmulPerfMode.DoubleRow

    nc = bacc.Bacc("TRN2", target_bir_lowering=False, debug=False,
                   num_devices=NCORES)
    r8d = nc.dram_tensor("r8d", [128, 2, 3, 66, 64], fp8, kind="ExternalInput")
    w1d = nc.dram_tensor("w1d", [128, 2, 9, C], fp8, kind="ExternalInput")
    w2d = nc.dram_tensor("w2d", [128, 2, 9, C], fp8, kind="ExternalInput")
    refd = nc.dram_tensor("refd", [128, 2, MHALF], f32, kind="ExternalInput")
    out = nc.dram_tensor("out", [C, MHALF], f32, kind="ExternalOutput")

    PIX = 66 * 64  # flat padded rows per (ic, dx)

    with tile.TileContext(nc) as tc:
        with tc.tile_pool(name="pers", bufs=1) as pers:
            r8 = pers.tile([128, 2, 3, PIX], fp8)
            w1r = pers.tile([128, 2, 9, C], fp8)
            w2r = pers.tile([128, 2, 9, C], fp8)
            x8 = pers.tile([128, 2, HW], fp8)
            vT8 = pers.tile([128, 32, C], fp8)
            dneg8 = pers.tile([1, 2, MHALF], fp8)
            wsrc = pers.tile([128, 512], fp8)
            nc.vector.memset(wsrc, 1.0)
            identf = pers.tile([128, 128], f32)
            make_identity(nc, identf)
            ones_r8 = pers.tile([1, 2, 128], fp8)
            nc.vector.memset(ones_r8, 1.0)
            ones_c8 = pers.tile([128, 2, 16], fp8)
            nc.vector.memset(ones_c8, 1.0)
            dmS = pers.tile([128, 4], f32)

            # ---------- input DMA ----------
            # conv1 needs w1 + all 6 r8 pieces first: w1 on SP, r8 pieces
            # split into row-halves round-robined so conv1 starts ~3us in.
            # w2 follows (needed in phase 2).
            nc.sync.dma_start(out=w1r[:, :, :, 0:128], in_=w1d[:, :, :, 0:128])
            nc.sync.dma_start(out=w1r[:, :, :, 128:256],
                              in_=w1d[:, :, :, 128:256])
            # r8 in row-range chunks spread over the 3 DMA-capable queues so
            # early rows land fast (conv1 blk b needs rows <= 8b+9).
            for (r0, r1), ic, eng in (
                    ((0, 10), 0, nc.gpsimd), ((0, 10), 1, nc.scalar),
                    ((10, 22), 0, nc.gpsimd), ((10, 22), 1, nc.scalar),
                    ((22, 34), 0, nc.gpsimd), ((22, 34), 1, nc.scalar),
                    ((34, 50), 0, nc.gpsimd), ((34, 50), 1, nc.sync),
                    ((50, 66), 0, nc.scalar), ((50, 66), 1, nc.sync)):
                eng.dma_start(
                    out=r8[:, ic, :, r0 * 64:r1 * 64],
                    in_=r8d[:, ic, :, r0:r1, :].rearrange(
                        "p a b c -> p a (b c)"))
            nc.gpsimd.dma_start(out=w2r, in_=w2d[:, :, :, :])

            def conv1_chunk(ps, cc, blk):
                for t in range(9):
                    dy, dx = t // 3 - 1, t % 3 - 1
                    off = (blk * 8 + dy + 1) * 64
                    nc.tensor.matmul(
                        ps, w1r[:, :, t, cc * 128:(cc + 1) * 128],
                        r8[:, :, dx + 1, off:off + 512],
                        start=(t == 0), stop=(t == 8), perf_mode=DR)

            with tc.tile_pool(name="ep", bufs=4) as ep, \
                 tc.tile_pool(name="fin", bufs=2) as fin:

                Es = [None] * NBLK

                def duo(pool, j, q):
                    # one 2-bank psum tile = scores for nt=2q,2q+1 vs m-block
                    # j; exp of both halves in a single ACT instruction.
                    mlo = j * 512
                    supd = pool.tile([128, 2, 512], f32, tag="sup",
                                     name="supd")
                    for h in (0, 1):
                        nt = 2 * q + h
                        nc.tensor.matmul(
                            supd[:, h, :],
                            x8[:, :, nt * 128:(nt + 1) * 128],
                            x8[:, :, mlo:mlo + 512],
                            start=True, stop=False, perf_mode=DR)
                        nc.tensor.matmul(
                            supd[:, h, :], ones_r8[0:1, :, :],
                            dneg8[0:1, :, mlo:mlo + 512],
                            start=False, stop=True, perf_mode=DR)
                    nc.scalar.activation(out=Es[j][:, 2 * q:2 * q + 2, :],
                                         in_=supd, func=AF.Exp)

                # ---------------- phase 1: conv1 + d + early scores -------
                with tc.tile_pool(name="cv1", bufs=4, space="PSUM") as cv1, \
                     tc.tile_pool(name="sup1", bufs=2,
                                  space="PSUM") as sup1, \
                     tc.tile_pool(name="sqp", bufs=4) as sqp:
                    # PE p-state warmup on memset data while input DMAs land
                    # (2.4 GHz needs 3us of continuous PE execution).
                    WARMUP_MMS = int(__import__('os').environ.get('WARM', '8'))
                    if WARMUP_MMS:
                        wps = cv1.tile([128, 512], f32, tag="cv", bufs=4,
                                       name="wps")
                        for _ in range(WARMUP_MMS):
                            nc.tensor.matmul(wps, wsrc[:, 0:128], wsrc,
                                             start=True, stop=True)

                    Es[0] = ep.tile([128, 32, 512], fp8, tag="E", name="E8")
                    Es[1] = ep.tile([128, 32, 512], fp8, tag="E", name="E8")
                    xsqs = [None] * NBLK

                    def d_chain_mm(j):
                        # d[m] = sum_c x8[c,m]^2 on PE: ones-column
                        # stationary vs xsq moving -> [1,512] psum, then the
                        # two exact fp8 bias pieces on DVE.
                        sl = slice(j * 512, (j + 1) * 512)
                        dps = cv1.tile([128, 512], f32, tag="cv", name="dps")
                        nc.tensor.matmul(dps[0:1, :], ones_c8[:, :, 0:1],
                                         xsqs[j], start=True, stop=True,
                                         perf_mode=DR)
                        nc.vector.tensor_scalar(
                            out=dneg8[0:1, 0, sl], in0=dps[0:1, :],
                            scalar1=-1.0, scalar2=None, op0=ALU.mult)
                        nc.vector.scalar_tensor_tensor(
                            out=dneg8[0:1, 1, sl], in0=dneg8[0:1, 0, sl],
                            scalar=-1.0, in1=dps[0:1, :],
                            op0=ALU.mult, op1=ALU.subtract)

                    # duo schedule: first duo needs dneg(block0) (ready
                    # ~1 blk after conv1 blk0), so duos start at blk2; the
                    # last 5 slots pull in j=1 duos to balance ACT vs PE.
                    PH1Q = {2: ((0, 0), (0, 1), (0, 2)),
                            3: ((0, 3), (0, 4), (0, 5)),
                            4: ((0, 6), (0, 7), (0, 8)),
                            5: ((0, 9), (0, 10), (0, 11), (1, 0)),
                            6: ((0, 12), (0, 13), (1, 1), (1, 2)),
                            7: ((0, 14), (0, 15),
                                (1, 3), (1, 4), (1, 5))}
                    for blk in range(8):
                        for cc in range(2):
                            if cc == 1 and blk in (1, 5, 6, 7):
                                d_chain_mm(0 if blk == 1 else blk - 4)
                            ps = cv1.tile([128, 512], f32, tag="cv",
                                          name="ps")
                            conv1_chunk(ps, cc, blk)
                            sl = slice(blk * 512, (blk + 1) * 512)
                            nc.vector.tensor_scalar(
                                out=x8[:, cc, sl], in0=ps,
                                scalar1=SCONV, scalar2=0.0,
                                op0=ALU.mult, op1=ALU.max)
                        if blk < NBLK:
                            # xsq: block 0 on DVE (d(0) latency gates the
                            # first duo), blocks 1-3 on Pool (idle, slack).
                            sl = slice(blk * 512, (blk + 1) * 512)
                            xsq = sqp.tile([128, 2, 512], fp8, tag="xsq",
                                           name="xsq")
                            nc.vector.tensor_mul(xsq, x8[:, :, sl],
                                                 x8[:, :, sl])
                            xsqs[blk] = xsq
                        for jq, q in PH1Q.get(blk, ()):
                            duo(sup1, jq, q)

                # ---------------- phase 2: conv2T + scores(j>=1) + A ------
                if True:

                    def c2unit(cv2t, pc):
                        # transposed conv2: out [128 pix, 256 cout] psum;
                        # gamma folded into the fp8 quantize scale.
                        ps2 = cv2t.tile([128, 256], f32, tag="c2", name="ps2")
                        for t in range(9):
                            dy, dx = t // 3 - 1, t % 3 - 1
                            off = (pc * 2 + dy + 1) * 64
                            nc.tensor.matmul(
                                ps2, r8[:, :, dx + 1, off:off + 128],
                                w2r[:, :, t, :],
                                start=(t == 0), stop=(t == 8), perf_mode=DR)
                        nc.vector.tensor_scalar(
                            out=vT8[:, pc, :], in0=ps2,
                            scalar1=SCONV * float(gamma), scalar2=0.0,
                            op0=ALU.mult, op1=ALU.max)

                    def a_cc(E8, pa, cc, t, st, sp):
                        nc.tensor.matmul(
                            pa,
                            vT8[:, 2 * t:2 * t + 2, cc * 128:(cc + 1) * 128],
                            E8[:, 2 * t:2 * t + 2, :],
                            start=st, stop=sp, perf_mode=DR)

                    def load_ref(jj, cc):
                        mlo = jj * 512
                        rf = fin.tile([128, 512], f32, tag=f"rf{cc}", bufs=2,
                                      name="rf")
                        nc.sync.dma_start(out=rf,
                                          in_=refd[:, cc, mlo:mlo + 512])
                        return rf

                    def finals_cc(jj, pa, bc, cc, rf, last):
                        # pa already carries gamma (folded into vT8); halves
                        # pipeline mult(DVE; Pool cannot read PSUM) ->
                        # add(Pool h0 / DVE h1, all-SBUF) -> DMA per 256 cols.
                        mlo = jj * 512
                        for h in (0, 1):
                            aeng = nc.vector if (last and h == 1) else nc.gpsimd
                            hs = slice(h * 256, (h + 1) * 256)
                            tmp = fin.tile([128, 256], f32, tag=f"tmp{cc}{h}",
                                           bufs=1, name="tmp")
                            nc.vector.tensor_mul(tmp, pa[:, hs], bc[:, hs])
                            ot = fin.tile([128, 256], f32, tag=f"ot{cc}{h}",
                                          bufs=2, name="ot")
                            aeng.tensor_add(ot, tmp, rf[:, hs])
                            nc.sync.dma_start(
                                out=out[cc * 128:(cc + 1) * 128,
                                        mlo + h * 256:mlo + (h + 1) * 256],
                                in_=ot)

                    def host_slot(jj, g, H, pool, dmw, dlo, last=False):
                        # A/D work of block jj, slot g (hosted inside a later
                        # scores loop or the drain). H: per-chain state;
                        # pool: psum pool for the pa accumulator; dmw/dlo:
                        # shared per-window D psum tile + column offset.
                        E8 = Es[jj]
                        if g == 0:
                            H['pa'] = pool.tile([128, 512], f32, tag="pa",
                                                name="pa")
                            H['rf0'] = load_ref(jj, 0)
                            H['rf1'] = load_ref(jj, 1)
                            # D[m] via tiny-out matmuls: E8 slice stationary,
                            # 1-col ones moving -> out [128 m-sub, 1].
                            for ms in range(4):
                                for t in range(16):
                                    nc.tensor.matmul(
                                        dmw[:, dlo + ms:dlo + ms + 1],
                                        E8[:, 2 * t:2 * t + 2,
                                           ms * 128:(ms + 1) * 128],
                                        ones_c8[:, :, 0:1],
                                        start=(t == 0), stop=(t == 15),
                                        perf_mode=DR)
                        elif g == 1:
                            dms = fin.tile([128, 4], f32, tag="dms", bufs=2,
                                           name="dms")
                            H['dms'] = dms
                            nc.vector.tensor_copy(
                                out=dms, in_=dmw[:, dlo:dlo + 4])
                        elif g == 2:
                            # 4 single-column transposes into partition-0
                            # rows of a sup-pool psum region (gpsimd bcast
                            # requires partition-0 APs)
                            tps = tpp.tile([128, 512], f32, tag="tps",
                                           name="tps")
                            H['tps'] = tps
                            for ms in range(4):
                                nc.tensor.transpose(
                                    tps[0:1, ms * 128:(ms + 1) * 128],
                                    H['dms'][:, ms:ms + 1], identf)
                        elif g == 3:
                            rp4 = fin.tile([1, 512], f32, tag="rp4", bufs=2,
                                           name="rp4")
                            H['rp4'] = rp4
                            nc.vector.reciprocal(
                                out=rp4, in_=H['tps'][0:1, 0:512])
                        elif g == 4:
                            bc = fin.tile([128, 512], f32, tag="bc",
                                          name="bc")
                            H['bc'] = bc
                            for ms in range(4):
                                nc.gpsimd.partition_broadcast(
                                    out_ap=bc[:, ms * 128:(ms + 1) * 128],
                                    in_ap=H['rp4'][0:1,
                                                   ms * 128:(ms + 1) * 128],
                                    channels=128)
                        if g < 8:
                            for t in (2 * g, 2 * g + 1):
                                a_cc(E8, H['pa'], 0, t,
                                     st=(t == 0), sp=(t == 15))
                        else:
                            if g == 8:
                                finals_cc(jj, H['pa'], H['bc'], 0, H['rf0'], last)
                                H['pa'] = pool.tile([128, 512], f32,
                                                    tag="pa", name="pa")
                            for t in (2 * (g - 8), 2 * (g - 8) + 1):
                                a_cc(E8, H['pa'], 1, t,
                                     st=(t == 0), sp=(t == 15))
                            if g == 15:
                                finals_cc(jj, H['pa'], H['bc'], 1, H['rf1'], last)

                    HA, HB = {}, {}
                    with tc.tile_pool(name="sup2", bufs=3,
                                      space="PSUM") as sup2, \
                         tc.tile_pool(name="cv2t", bufs=2,
                                      space="PSUM") as cv2t:
                        # j=1: duos q=6..15 (q<6 ran in phase 1) + 10 conv2T
                        # units (1/slot keeps these slots ACT-bound)
                        for i, q in enumerate(range(6, 16)):
                            duo(sup2, 1, q)
                            c2unit(cv2t, i)
                        # j=2: scores + remaining 22 conv2T units; no A
                        # hosting here (j2 is already PE-heavy)
                        Es[2] = ep.tile([128, 32, 512], fp8, tag="E",
                                        name="E8")
                        pc = 10
                        for g in range(16):
                            duo(sup2, 2, g)
                            for _ in range(2 if g < 6 else 1):
                                c2unit(cv2t, pc)
                                pc += 1
                    with tc.tile_pool(name="sup3", bufs=2,
                                      space="PSUM") as sup3, \
                         tc.tile_pool(name="pap", bufs=1,
                                      space="PSUM") as pap, \
                         tc.tile_pool(name="pa2", bufs=1,
                                      space="PSUM") as pa2, \
                         tc.tile_pool(name="tpp", bufs=1,
                                      space="PSUM") as tpp, \
                         tc.tile_pool(name="dmp", bufs=1,
                                      space="PSUM") as dmp:
                        # j=3: scores + A(0) and A(1) as dual chains (all E
                        # tiles and vT ready; these slots are ACT-paced)
                        Es[3] = ep.tile([128, 32, 512], fp8, tag="E",
                                        name="E8")
                        dmw = dmp.tile([128, 8], f32, tag="dm4", name="dmw")
                        for g in range(16):
                            duo(sup3, 3, g)
                            host_slot(0, g, HA, pap, dmw, 0)
                            host_slot(1, g, HB, pa2, dmw, 4)
                        # drain. Ordered for continuous PE flow (any
                        # long PE idle resets the p-state and the tail then
                        # runs at 0.65 GHz): cc0 of A(2) then A(3) back to
                        # back, then the D chains and the cc1 halves. The
                        # freed sup3 tiles serve as the cc1 accumulators.
                        dmw = dmp.tile([128, 8], f32, tag="dm4", name="dmw")

                        def dmm64(E8, dlo):
                            for ms in range(4):
                                for t in range(16):
                                    nc.tensor.matmul(
                                        dmw[:, dlo + ms:dlo + ms + 1],
                                        E8[:, 2 * t:2 * t + 2,
                                           ms * 128:(ms + 1) * 128],
                                        ones_c8[:, :, 0:1],
                                        start=(t == 0), stop=(t == 15),
                                        perf_mode=DR)

                        def dchain(dlo):
                            dms = fin.tile([128, 4], f32, tag="dms", bufs=2,
                                           name="dms")
                            nc.vector.tensor_copy(out=dms,
                                                  in_=dmw[:, dlo:dlo + 4])
                            tps = tpp.tile([128, 512], f32, tag="tps",
                                           name="tps")
                            for ms in range(4):
                                nc.tensor.transpose(
                                    tps[0:1, ms * 128:(ms + 1) * 128],
                                    dms[:, ms:ms + 1], identf)
                            rp4 = fin.tile([1, 512], f32, tag="rp4", bufs=2,
                                           name="rp4")
                            nc.vector.reciprocal(out=rp4, in_=tps[0:1, 0:512])
                            bc = fin.tile([128, 512], f32, tag="bc",
                                          name="bc")
                            for ms in range(4):
                                nc.gpsimd.partition_broadcast(
                                    out_ap=bc[:, ms * 128:(ms + 1) * 128],
                                    in_ap=rp4[0:1, ms * 128:(ms + 1) * 128],
                                    channels=128)
                            return bc

                        # A(2) start to finish (E(2) complete -> full
                        # speed, overlapping the last exps of E(3))
                        pa2c0 = pap.tile([128, 512], f32, tag="pa",
                                         name="pa")
                        rf20, rf21 = load_ref(2, 0), load_ref(2, 1)
                        for t in range(16):
                            a_cc(Es[2], pa2c0, 0, t,
                                 st=(t == 0), sp=(t == 15))
                        dmm64(Es[2], 0)
                        bc2 = dchain(0)
                        s3a = sup3.tile([128, 2, 512], f32, tag="sup",
                                        name="s3a")
                        pa2c1 = s3a[:, 0, :]
                        for t in range(16):
                            a_cc(Es[2], pa2c1, 1, t,
                                 st=(t == 0), sp=(t == 15))
                            if t == 3:
                                finals_cc(2, pa2c0, bc2, 0, rf20, False)
                        finals_cc(2, pa2c1, bc2, 1, rf21, False)
                        # A(3), head paced by the tail of the exp stream
                        pa3c0 = pa2.tile([128, 512], f32, tag="pa",
                                         name="pa")
                        rf30, rf31 = load_ref(3, 0), load_ref(3, 1)
                        for t in range(16):
                            a_cc(Es[3], pa3c0, 0, t,
                                 st=(t == 0), sp=(t == 15))
                        dmm64(Es[3], 4)
                        bc3 = dchain(4)
                        s3b = sup3.tile([128, 2, 512], f32, tag="sup",
                                        name="s3b")
                        pa3c1 = s3b[:, 0, :]
                        for t in range(16):
                            a_cc(Es[3], pa3c1, 1, t,
                                 st=(t == 0), sp=(t == 15))
                            if t == 3:
                                finals_cc(3, pa3c0, bc3, 0, rf30, True)
                        finals_cc(3, pa3c1, bc3, 1, rf31, True)

    nc.compile()
    return nc


def _make_runner(nc):
    import jax
    from jax.sharding import Mesh, PartitionSpec
    from jax.experimental.shard_map import shard_map
    import concourse.mybir as mybir
    from concourse.bass2jax import (_bass_exec_p, install_neuronx_cc_hook,
                                    partition_id_tensor)

    install_neuronx_cc_hook()
    partition_name = (nc.partition_id_tensor.name
                      if nc.partition_id_tensor else None)
    in_names, out_names, out_avals, zero_outs = [], [], [], []
    for alloc in nc.m.functions[0].allocations:
        if not isinstance(alloc, mybir.MemoryLocationSet):
            continue
        name = alloc.memorylocations[0].name
        if alloc.kind == "ExternalInput":
            if name != partition_name:
                in_names.append(name)
        elif alloc.kind == "ExternalOutput":
            shape = tuple(alloc.tensor_shape)
            dtype = mybir.dt.np(alloc.dtype)
            out_avals.append(jax.core.ShapedArray(shape, dtype))
            out_names.append(name)
            zero_outs.append(np.zeros(shape, dtype))
    n_params = len(in_names)
    n_outs = len(out_avals)
    all_in_names = list(in_names) + list(out_names)
    if partition_name is not None:
        all_in_names.append(partition_name)

    def _body(*args):
        operands = list(args)
        if partition_name is not None:
            operands.append(partition_id_tensor())
        return tuple(_bass_exec_p.bind(
            *operands, out_avals=tuple(out_avals),
            in_names=tuple(all_in_names), out_names=tuple(out_names),
            lowering_input_output_aliases=(),
            sim_require_finite=True, sim_require_nnan=True, nc=nc))

    devices = jax.devices()[:NCORES]
    mesh = Mesh(np.asarray(devices), ("core",))
    jitted = jax.jit(
        shard_map(_body, mesh=mesh,
                  in_specs=(PartitionSpec("core"),) * (n_params + n_outs),
                  out_specs=(PartitionSpec("core"),) * n_outs,
                  check_rep=False),
        keep_unused=True)

    def run(in_maps):
        import jax as _jax
        per_core = [[np.asarray(m[n]) for n in in_names] for m in in_maps]
        concat_in = [
            np.ascontiguousarray(
                np.concatenate([per_core[c][i] for c in range(NCORES)],
                               axis=0))
            for i in range(n_params)
        ]
        concat_zeros = [
            np.zeros((NCORES * z.shape[0], *z.shape[1:]), z.dtype)
            for z in zero_outs
        ]
        outs = jitted(*concat_in, *concat_zeros)
        _jax.block_until_ready(outs)
        return [
            {n: np.asarray(outs[i]).reshape(NCORES, *out_avals[i].shape)[c]
             for i, n in enumerate(out_names)}
            for c in range(NCORES)
        ]

    return run


def _prep_weights(w):
    # w: [O, I, 3, 3] -> [128, 2, 9, 256] fp8 ([cin128, ic, tap, cout]), scaled
    wt = np.transpose(w, (1, 2, 3, 0)).reshape(C, 9, C)
    wt = wt.reshape(2, 128, 9, C).transpose(1, 0, 2, 3)
    return np.ascontiguousarray((SW * wt).astype(F8NP))


def _prep_ref(r):
    # r: [C, H, W] f32 -> [128, 2, 3, 66, 64] fp8, scaled by SR
    rp = np.zeros((C, H + 2, W + 2), np.float32)
    rp[:, 1:H + 1, 1:W + 1] = SR * r
    r8 = rp.astype(F8NP)
    o = np.empty((C, 3, 66, 64), F8NP)
    for dxi, dx in enumerate((-1, 0, 1)):
        o[:, dxi] = r8[:, :, 1 + dx:65 + dx]
    return np.ascontiguousarray(
        o.reshape(2, 128, 3, 66, 64).transpose(1, 0, 2, 3, 4))


def make_in_maps(inputs_np, ref_np, w1_np, w2_np):
    w18 = _prep_weights(w1_np)
    w28 = _prep_weights(w2_np)
    w18r = _prep_weights(w1_np[:, :, ::-1, ::-1])
    w28r = _prep_weights(w2_np[:, :, ::-1, ::-1])
    in_maps = []
    for core in range(NCORES):
        b, rot = core // 2, core % 2
        r = ref_np[b]
        if rot:
            r = r[:, ::-1, ::-1]
        r = np.ascontiguousarray(r)
        reff = np.ascontiguousarray(
            r.reshape(C, HW)[:, :MHALF].reshape(2, 128, MHALF)
            .transpose(1, 0, 2)).astype(np.float32)
        in_maps.append({
            "r8d": _prep_ref(r),
            "w1d": w18r if rot else w18,
            "w2d": w28r if rot else w28,
            "refd": reff,
        })
    return in_maps


def assemble(results, ref_np, gamma):
    full = np.empty((B, C, HW), np.float32)
    for core in range(NCORES):
        b, rot = core // 2, core % 2
        o = results[core]["out"]
        if rot:
            full[b][:, MHALF:] = o[:, ::-1]
        else:
            full[b][:, :MHALF] = o
    return full.reshape(B, C, HW).reshape(B, C, H, W)


def kernel(inputs, ref, w1, w2, gamma):
    inputs = np.asarray(inputs, np.float32)
    ref = np.asarray(ref, np.float32)
    w1 = np.asarray(w1, np.float32)
    w2 = np.asarray(w2, np.float32)
    g = float(np.asarray(gamma))
    key = ("k", g)
    if key not in _CACHE:
        nc = _build(g)
        _CACHE[("nc", g)] = nc
        _CACHE[key] = _make_runner(nc)
    run = _CACHE[key]
    in_maps = make_in_maps(inputs, ref, w1, w2)
    results = run(in_maps)
    return assemble(results, ref, g)
